# revision 2
# baseline (speedup 1.0000x reference)
"""Bahdanau (additive) attention kernel for 8 Trainium2 NeuronCores.

Problem:  hidden [1,16,512], encoder_outputs [4096,16,512], W [512,1024],
          b [512], v [512]  ->  ctx [1,16,512]
  energy = tanh(hidden @ Wh.T + enc @ We.T + b); scores = energy @ v
  attn = softmax_T(scores); ctx = sum_t attn * enc
  rel-err budget 2e-2; all-bf16 pipeline measures 2.3e-3.

Sharding: data-parallel over batch B=16 -> 2 batches per core, no comm.

Key layout decisions (v2):
  - enc is cast to bf16 ON HOST and packed batch-major [BL, T, H]: halves
    HBM traffic vs f32 (8.4MB/core, ~24us at ~358GB/s) and removes the
    on-device f32->bf16 DVE cast entirely. Slab loads are contiguous 1MB
    SWDGE DMAs straight into the resident natural-layout tile xn.
  - ONE xbar DMA-transpose per 1024-row slab -> h-on-partitions layout
    [p, n, hc, r]; the energy matmul reads it with a strided rhs AP.
  - energyT psum tiles [128 h_out, 1024 t] = WeT.T @ XT  (PE, bf16)
  - tanh on ACT with per-partition bias = (hidden @ Wh.T + b)  (folded)
  - scores = v.T @ tanhE on PE -> psum [1, 512] blocks
  - exp on ACT (unnormalized softmax; fp32 range is safe since
    |scores| <= ||v||_1 ~ 18) with fused accum_out giving block sums
  - attn row [1,T] -> [128 t, 32] layout via padded xbar transpose
  - ctx_raw = sum_t attn_t * X_t on PE (natural layout), then scale by
    1/sum(exp) on DVE, DMA out.
"""

import sys

if "/opt/trn_rl_repo" not in sys.path:
    sys.path.insert(0, "/opt/trn_rl_repo")

import ml_dtypes
import numpy as np

import concourse.bass as bass
import concourse.tile as tile
from concourse import bacc, mybir

F32 = mybir.dt.float32
BF16 = mybir.dt.bfloat16

N_CORES = 8
T = 4096
B = 16
H = 512
BL = B // N_CORES  # batches per core = 2
HC = H // 128  # h chunks = 4
TB = 1024  # t macro-block (slab)
NC32 = T // 128  # 32 t-chunks of 128 per batch


def build_nc(
    n_iters: int = 1,
    debug: bool = False,
    t_dim: int = T,
    loop_iters: int = 0,
    bufs: dict | None = None,
    stagger: bool = True,
    **_unused,
):
    """Build the per-core Bass program. n_iters>1 unrolls the whole main
    body; loop_iters>0 additionally wraps the body in a hardware For_i
    loop (for wall-clock benchmarking -- the axon transport adds tens of
    ms of jitter, so device time must dominate)."""
    bufs = dict(bufs or {})

    def nb(name, default):
        return bufs.get(name, default)

    t = t_dim
    nc32 = t // 128
    nblk = t // 512  # exp blocks per batch

    nc = bacc.Bacc(
        "TRN2", target_bir_lowering=False, debug=debug, num_devices=N_CORES
    )
    enc_d = nc.declare_dram_parameter("enc", [BL, t, H], BF16, isOutput=False)
    wt_d = nc.declare_dram_parameter("wt", [2 * H, H], F32, isOutput=False)
    ht_d = nc.declare_dram_parameter("ht", [128, HC, BL], F32, isOutput=False)
    b_d = nc.declare_dram_parameter("bias", [128, HC], F32, isOutput=False)
    v_d = nc.declare_dram_parameter("v", [128, HC], F32, isOutput=False)
    out_d = nc.declare_dram_parameter("out", [1, BL, H], F32, isOutput=True)

    with tile.TileContext(nc) as tc:
        with tc.tile_pool(name="persist", bufs=1) as persist:
            # ---- one-time setup: weights, query bias ----
            wet_bf = persist.tile([128, HC, H], BF16, tag="wet")
            qb_sb = persist.tile([128, HC, BL], F32, tag="qb")
            v_bf = persist.tile([128, HC], BF16, tag="vbf")
            srow_p = [
                persist.tile([16, t], BF16, tag=f"srow{i}", name=f"srow{i}")
                for i in range(BL)
            ]
            for i in range(BL):
                nc.gpsimd.memset(srow_p[i][:], 0.0)
            with (
                tc.tile_pool(name="setup", bufs=1) as setup,
                tc.tile_pool(
                    name="setup_ps", bufs=1, space=bass.MemorySpace.PSUM
                ) as setup_ps,
            ):
                # setup loads go on the ACT HWDGE ring (idle at start)
                # so the first enc transpose isn't queued behind them on
                # the sync ring; Wh half first so the q-matmul (first in
                # the PE stream) unblocks after 1MB instead of 2MB
                ht_sb = setup.tile([128, HC, BL], F32, tag="ht")
                nc.scalar.dma_start(ht_sb[:], ht_d[:])
                b_sb = setup.tile([128, HC], F32, tag="bias")
                nc.scalar.dma_start(b_sb[:], b_d[:])
                v_f32 = setup.tile([128, HC], F32, tag="vf")
                nc.scalar.dma_start(v_f32[:], v_d[:])
                wt_sb = setup.tile([128, 2 * HC, H], F32, tag="wt")
                wt_r = wt_d.rearrange("(c p) i -> p c i", p=128)
                nc.scalar.dma_start(wt_sb[:, :HC, :], wt_r[:, :HC, :])
                nc.scalar.dma_start(wt_sb[:, HC:, :], wt_r[:, HC:, :])
                nc.vector.tensor_copy(v_bf[:], v_f32[:])
                for jc in range(HC):
                    nc.vector.tensor_copy(
                        wet_bf[:, jc, :], wt_sb[:, HC + jc, :]
                    )
                # q[i, b] = sum_j hidden[b, j] W[i, j]  (Wh half of Wt)
                q_ps = setup_ps.tile([128, HC, BL], F32, tag="qps")
                for ic in range(HC):
                    for jc in range(HC):
                        nc.tensor.matmul(
                            q_ps[:, ic, :],
                            wt_sb[:, jc, bass.ts(ic, 128)],
                            ht_sb[:, jc, :],
                            start=(jc == 0),
                            stop=(jc == HC - 1),
                        )
                # qb = q + bias  (per-partition bias add on ACT)
                for ic in range(HC):
                    nc.scalar.activation(
                        qb_sb[:, ic, :],
                        q_ps[:, ic, :],
                        mybir.ActivationFunctionType.Identity,
                        bias=b_sb[:, ic : ic + 1],
                    )

            # ---- main pools ----
            with (
                tc.tile_pool(name="xn", bufs=nb("xn", 2)) as xn_pool,
                tc.tile_pool(name="xt", bufs=nb("xt", 4)) as xt_pool,
                tc.tile_pool(name="tanh", bufs=nb("tanh", 6)) as tanh_pool,
                tc.tile_pool(name="small", bufs=nb("small", 3)) as small_pool,
                tc.tile_pool(
                    name="eps", bufs=nb("eps", 2), space=bass.MemorySpace.PSUM
                ) as e_ps_pool,
                tc.tile_pool(
                    name="sps", bufs=nb("sps", 2), space=bass.MemorySpace.PSUM
                ) as s_ps_pool,
                tc.tile_pool(
                    name="cps", bufs=nb("cps", 2), space=bass.MemorySpace.PSUM
                ) as c_ps_pool,
            ):

                def slab(xn, srow, sums, bi, h0, nh):
                    # load nh*512 bf16 rows starting at 512*h0 straight
                    # into the resident natural-layout tile (contiguous
                    # 1MB SWDGE DMA; enc is batch-major on host)
                    nc.gpsimd.dma_start(
                        xn[:, h0 * 4 : (h0 + nh) * 4, :],
                        enc_d[bi, h0 * 512 : (h0 + nh) * 512, :].rearrange(
                            "(n p) h -> p n h", p=128
                        ),
                    )
                    # transpose to h-on-partitions in ONE xbar op per
                    # slab: out[p, n, hc, r] = xn[r, n, hc*128+p].  Big ops
                    # measured 200 GB/s vs 127 for per-chunk ops.
                    xt = xt_pool.tile([128, nh * 4, HC, 128], BF16, tag="xt")
                    nc.sync.dma_start(
                        xt[:],
                        xn[:, h0 * 4 : (h0 + nh) * 4, :],
                        transpose=True,
                    )
                    # energyT[ho, t] += WeT.T @ XT ; tanh ; v-reduce ; exp
                    s_ps = [
                        s_ps_pool.tile([1, 512], F32, tag="sps", name=f"sps{h2}")
                        for h2 in range(nh)
                    ]
                    ths = []
                    for ic in range(HC):
                        e_ps = e_ps_pool.tile([128, nh * 512], F32, tag="eps")
                        for jc in range(HC):
                            for half in range(nh):
                                sl = bass.ts(half, 512)
                                nc.tensor.matmul(
                                    e_ps[:, sl],
                                    wet_bf[:, jc, bass.ts(ic, 128)],
                                    xt[:, bass.ts(half, 4), jc, :],
                                    start=(jc == 0),
                                    stop=(jc == HC - 1),
                                )
                        th = tanh_pool.tile(
                            [128, nh * 512], BF16, tag="th", name=f"th{ic}"
                        )
                        nc.scalar.activation(
                            th[:],
                            e_ps[:],
                            mybir.ActivationFunctionType.Tanh,
                            bias=qb_sb[:, ic, bi : bi + 1],
                        )
                        ths.append(th)
                    for ic in range(HC):
                        for half in range(nh):
                            sl = bass.ts(half, 512)
                            nc.tensor.matmul(
                                s_ps[half][:],
                                v_bf[:, ic : ic + 1],
                                ths[ic][:, sl],
                                start=(ic == 0),
                                stop=(ic == HC - 1),
                            )
                    for half in range(nh):
                        blk = h0 + half
                        nc.scalar.activation(
                            srow[0:1, bass.ts(blk, 512)],
                            s_ps[half][:],
                            mybir.ActivationFunctionType.Exp,
                            accum_out=sums[0:1, blk : blk + 1],
                        )

                def finish_softmax(srow, sums):
                    attn_t = small_pool.tile([128, nc32, 16], BF16, tag="attn")
                    nc.sync.dma_start(attn_t[:], srow[:], transpose=True)
                    s_tot = small_pool.tile([1, 1], F32, tag="stot")
                    nc.vector.reduce_sum(
                        s_tot[:], sums[:], axis=mybir.AxisListType.X
                    )
                    r_s = small_pool.tile([1, 1], F32, tag="rs")
                    nc.vector.reciprocal(r_s[:], s_tot[:])
                    return attn_t, r_s

                def ctx_phase(xn, attn_t, r_s, bi):
                    c_ps = c_ps_pool.tile([1, H], F32, tag="cps")
                    for c in range(nc32):
                        nc.tensor.matmul(
                            c_ps[:],
                            attn_t[:, c, 0:1],
                            xn[:, c, :],
                            start=(c == 0),
                            stop=(c == nc32 - 1),
                        )
                    ctx_sb = small_pool.tile([1, H], F32, tag="ctx")
                    nc.vector.tensor_scalar_mul(ctx_sb[:], c_ps[:], r_s[:])
                    nc.sync.dma_start(out_d[:, bi, :], ctx_sb[:])

                def main_body():
                    for _ in range(n_iters):
                        xns = [
                            xn_pool.tile(
                                [128, nc32, H], BF16, tag="xn", name=f"xn{i}"
                            )
                            for i in range(BL)
                        ]
                        sums_l = [
                            small_pool.tile(
                                [1, nblk], F32, tag="sums", name=f"sums{i}"
                            )
                            for i in range(BL)
                        ]
                        # 1024-row units; batch 1 trails batch 0 by one
                        # slab so the DMA stream never drains
                        u0 = [(i, 2) for i in range(0, nblk, 2)]
                        u1 = list(u0)
                        if stagger:
                            slab(xns[0], srow_p[0], sums_l[0], 0, *u0[0])
                            for i in range(1, len(u0)):
                                slab(xns[0], srow_p[0], sums_l[0], 0, *u0[i])
                                slab(xns[1], srow_p[1], sums_l[1], 1, *u1[i - 1])
                            at0, rs0 = finish_softmax(srow_p[0], sums_l[0])
                            slab(xns[1], srow_p[1], sums_l[1], 1, *u1[-1])
                            ctx_phase(xns[0], at0, rs0, 0)
                            at1, rs1 = finish_softmax(srow_p[1], sums_l[1])
                            ctx_phase(xns[1], at1, rs1, 1)
                        else:
                            for bi in range(BL):
                                for u in (u0, u1)[bi]:
                                    slab(
                                        xns[bi], srow_p[bi], sums_l[bi], bi, *u
                                    )
                                at, rs = finish_softmax(
                                    srow_p[bi], sums_l[bi]
                                )
                                ctx_phase(xns[bi], at, rs, bi)

                if loop_iters > 0:
                    with tc.For_i(0, loop_iters, 1):
                        main_body()
                else:
                    main_body()

    nc.compile()
    return nc


# ---------------------------------------------------------------------------
# Host side: shard, run on 8 cores via PJRT (axon), gather.
# ---------------------------------------------------------------------------


class _SpmdRunner:
    """Build the sharded jit once; reuse across calls."""

    def __init__(self, nc, n_cores):
        import jax
        from jax.sharding import Mesh, PartitionSpec
        from jax.experimental.shard_map import shard_map
        from concourse.bass2jax import (
            _bass_exec_p,
            install_neuronx_cc_hook,
            partition_id_tensor,
        )

        install_neuronx_cc_hook()
        self.jax = jax
        self.n_cores = n_cores
        partition_name = (
            nc.partition_id_tensor.name if nc.partition_id_tensor else None
        )
        in_names, out_names, out_avals, zero_outs = [], [], [], []
        for alloc in nc.m.functions[0].allocations:
            if not isinstance(alloc, mybir.MemoryLocationSet):
                continue
            name = alloc.memorylocations[0].name
            if alloc.kind == "ExternalInput":
                if name != partition_name:
                    in_names.append(name)
            elif alloc.kind == "ExternalOutput":
                shape = tuple(alloc.tensor_shape)
                dtype = mybir.dt.np(alloc.dtype)
                out_names.append(name)
                out_avals.append(jax.core.ShapedArray(shape, dtype))
                zero_outs.append(np.zeros(shape, dtype))
        self.in_names = list(in_names)
        self.out_names = out_names
        self.out_avals = out_avals
        n_params = len(in_names)
        n_outs = len(out_avals)
        all_in_names = in_names + out_names
        if partition_name is not None:
            all_in_names.append(partition_name)

        def _body(*args):
            operands = list(args)
            if partition_name is not None:
                operands.append(partition_id_tensor())
            outs = _bass_exec_p.bind(
                *operands,
                out_avals=tuple(out_avals),
                in_names=tuple(all_in_names),
                out_names=tuple(out_names),
                lowering_input_output_aliases=(),
                sim_require_finite=True,
                sim_require_nnan=True,
                nc=nc,
            )
            return tuple(outs)

        devices = jax.devices()[:n_cores]
        assert len(devices) == n_cores, (
            f"need {n_cores} neuron cores, found {len(jax.devices())}"
        )
        self.mesh = Mesh(np.asarray(devices), ("core",))
        in_specs = (PartitionSpec("core"),) * (n_params + n_outs)
        out_specs = (PartitionSpec("core"),) * n_outs
        self.fn = jax.jit(
            shard_map(
                _body,
                mesh=self.mesh,
                in_specs=in_specs,
                out_specs=out_specs,
                check_rep=False,
            ),
            keep_unused=True,
        )
        self._sharding = jax.sharding.NamedSharding(
            self.mesh, PartitionSpec("core")
        )
        self._dev_zeros = [
            jax.device_put(
                np.zeros((n_cores * z.shape[0], *z.shape[1:]), z.dtype),
                self._sharding,
            )
            for z in zero_outs
        ]

    def put_inputs(self, in_maps):
        n = self.n_cores
        concat_in = [
            np.concatenate(
                [np.asarray(in_maps[c][name]) for c in range(n)], axis=0
            )
            for name in self.in_names
        ]
        return [self.jax.device_put(x, self._sharding) for x in concat_in]

    def run_device(self, dev_in):
        outs = self.fn(*dev_in, *self._dev_zeros)
        self.jax.block_until_ready(outs)
        return outs

    def run(self, in_maps):
        outs = self.run_device(self.put_inputs(in_maps))
        n = self.n_cores
        return [
            {
                name: np.asarray(outs[i]).reshape(
                    n, *self.out_avals[i].shape
                )[c]
                for i, name in enumerate(self.out_names)
            }
            for c in range(n)
        ]


def shard_inputs(hidden, encoder_outputs, W, b, v):
    """Per-core input dicts. Big tensor is sliced, cast to bf16 and made
    batch-major; small tensors are layout-packed for 128-partition SBUF
    residency."""
    hidden = np.asarray(hidden, dtype=np.float32)
    enc = np.asarray(encoder_outputs, dtype=np.float32)
    W = np.asarray(W, dtype=np.float32)
    b = np.asarray(b, dtype=np.float32)
    v = np.asarray(v, dtype=np.float32)

    wt = np.ascontiguousarray(W.T)  # [2H, H]
    b_packed = np.ascontiguousarray(b.reshape(HC, 128).T)  # [128, HC]
    v_packed = np.ascontiguousarray(v.reshape(HC, 128).T)  # [128, HC]
    in_maps = []
    for c in range(N_CORES):
        bsl = slice(c * BL, (c + 1) * BL)
        h_l = hidden[0, bsl, :]  # [BL, H]
        # ht_packed[p, jc, b] = hidden[b, jc*128 + p]
        ht_packed = np.ascontiguousarray(
            h_l.T.reshape(HC, 128, BL).transpose(1, 0, 2)
        )
        # batch-major bf16 enc slice [BL, T, H]
        enc_bm = np.ascontiguousarray(
            enc[:, bsl, :].transpose(1, 0, 2)
        ).astype(ml_dtypes.bfloat16)
        in_maps.append(
            {
                "enc": enc_bm,
                "wt": wt,
                "ht": ht_packed,
                "bias": b_packed,
                "v": v_packed,
            }
        )
    return in_maps


_RUNNER = None


def _get_runner():
    global _RUNNER
    if _RUNNER is None:
        _RUNNER = _SpmdRunner(build_nc(), N_CORES)
    return _RUNNER


def kernel(hidden, encoder_outputs, W, b, v):
    runner = _get_runner()
    in_maps = shard_inputs(hidden, encoder_outputs, W, b, v)
    res = runner.run(in_maps)
    out = np.concatenate([r["out"] for r in res], axis=1)  # [1, 16, 512]
    return out.astype(np.float32)


# revision 9
# speedup vs baseline: 1.9054x; 1.9054x over previous
"""Bahdanau (additive) attention kernel for 8 Trainium2 NeuronCores.

Problem:  hidden [1,16,512], encoder_outputs [4096,16,512], W [512,1024],
          b [512], v [512]  ->  ctx [1,16,512]
  energy = tanh(hidden @ Wh.T + enc @ We.T + b); scores = energy @ v
  attn = softmax_T(scores); ctx = sum_t attn * enc
  rel-err budget 2e-2; all-bf16 pipeline measures 2.3e-3.

Sharding: data-parallel over batch B=16 -> 2 batches per core, no comm.

Key layout decisions (v3):
  - enc is cast to bf16 ON HOST and staged in DRAM TWICE: natural
    [T, BL, H] (feeds the ctx matmul, which contracts over t) and
    pre-transposed [H, BL, T] (feeds the energy matmul, which contracts
    over h).  This removes the on-device xbar transpose, which measured
    68us/iter and serialized against the HBM loads.  Total HBM traffic
    16.8MB/core (~47us at 358GB/s) stays under the bf16 PE floor (82us),
    so the kernel is PE-bound as it should be.
  - transposed loads go on the sync HWDGE ring (PE-critical path);
    natural loads trickle on the gpsimd SWDGE ring (only needed by the
    ctx matmuls at the very end).
  - energyT psum tiles [128 h_out, 1024 t] = WeT.T @ XT  (PE, bf16,
    1024-col moving operand = max for bf16)
  - tanh on ACT with per-partition bias = (hidden @ Wh.T + b)  (folded)
  - scores = v.T @ tanhE on PE -> psum [1, 512] blocks
  - exp on ACT (unnormalized softmax; fp32 range is safe since
    |scores| <= ||v||_1 ~ 18) with fused accum_out giving block sums
  - attn row [1,T] -> [128 t, 32] layout via padded xbar transpose
  - ctx_raw = sum_t attn_t * X_t on PE (natural layout), then scale by
    1/sum(exp) on DVE, DMA out.
"""

import sys

if "/opt/trn_rl_repo" not in sys.path:
    sys.path.insert(0, "/opt/trn_rl_repo")

import ml_dtypes
import numpy as np

import concourse.bass as bass
import concourse.tile as tile
from concourse import bacc, mybir

F32 = mybir.dt.float32
BF16 = mybir.dt.bfloat16

N_CORES = 8
T = 4096
B = 16
H = 512
BL = B // N_CORES  # batches per core = 2
HC = H // 128  # h chunks = 4
TB = 1024  # t macro-block (slab)
NC32 = T // 128  # 32 t-chunks of 128 per batch


def build_nc(
    n_iters: int = 1,
    debug: bool = False,
    t_dim: int = T,
    loop_iters: int = 0,
    bufs: dict | None = None,
    # bisection knob: subset of {"load", "energy", "tanh", "vdot",
    # "finish"} - later stages require earlier ones. None = all.
    stages: frozenset | None = None,
    **_unused,
):
    """Build the per-core Bass program. n_iters>1 unrolls the whole main
    body; loop_iters>0 additionally wraps the body in a hardware For_i
    loop (for wall-clock benchmarking -- the axon transport adds tens of
    ms of jitter, so device time must dominate)."""
    bufs = dict(bufs or {})

    def nb(name, default):
        return bufs.get(name, default)

    t = t_dim
    ns = t // TB  # number of slabs
    nblk = t // 512  # exp blocks per batch

    nc = bacc.Bacc(
        "TRN2", target_bir_lowering=False, debug=debug, num_devices=N_CORES
    )
    enc_d = nc.declare_dram_parameter("enc", [t, BL, H], BF16, isOutput=False)
    enct_d = nc.declare_dram_parameter(
        "enct", [H, BL, t], BF16, isOutput=False
    )
    wt_d = nc.declare_dram_parameter("wt", [2 * H, H], F32, isOutput=False)
    ht_d = nc.declare_dram_parameter("ht", [128, HC, BL], F32, isOutput=False)
    b_d = nc.declare_dram_parameter("bias", [128, HC], F32, isOutput=False)
    v_d = nc.declare_dram_parameter("v", [128, HC], F32, isOutput=False)
    out_d = nc.declare_dram_parameter("out", [1, BL, H], F32, isOutput=True)

    ST = stages

    def on(s):
        return ST is None or s in ST

    with tile.TileContext(nc) as tc:
        with tc.tile_pool(name="persist", bufs=1) as persist:
            # ---- one-time setup: weights, query bias ----
            wet_bf = persist.tile([128, HC, H], BF16, tag="wet")
            qb_sb = persist.tile([128, HC, BL], F32, tag="qb")
            v_bf = persist.tile([128, HC], BF16, tag="vbf")
            srow_p = [
                persist.tile([16, t], BF16, tag=f"srow{i}", name=f"srow{i}")
                for i in range(BL)
            ]
            for i in range(BL):
                nc.gpsimd.memset(srow_p[i][:], 0.0)
            with (
                tc.tile_pool(name="setup", bufs=1) as setup,
                tc.tile_pool(
                    name="setup_ps", bufs=1, space=bass.MemorySpace.PSUM
                ) as setup_ps,
            ):
                # setup loads go on the ACT HWDGE ring so the first enc
                # loads aren't queued behind them on the sync ring; Wh
                # half first so the q-matmul (first in the PE stream)
                # unblocks after 1MB instead of 2MB
                ht_sb = setup.tile([128, HC, BL], F32, tag="ht")
                nc.scalar.dma_start(ht_sb[:], ht_d[:])
                b_sb = setup.tile([128, HC], F32, tag="bias")
                nc.scalar.dma_start(b_sb[:], b_d[:])
                v_f32 = setup.tile([128, HC], F32, tag="vf")
                nc.scalar.dma_start(v_f32[:], v_d[:])
                wt_sb = setup.tile([128, 2 * HC, H], F32, tag="wt")
                wt_r = wt_d.rearrange("(c p) i -> p c i", p=128)
                nc.scalar.dma_start(wt_sb[:, :HC, :], wt_r[:, :HC, :])
                nc.scalar.dma_start(wt_sb[:, HC:, :], wt_r[:, HC:, :])
                nc.vector.tensor_copy(v_bf[:], v_f32[:])
                for jc in range(HC):
                    nc.vector.tensor_copy(
                        wet_bf[:, jc, :], wt_sb[:, HC + jc, :]
                    )
                # q[i, b] = sum_j hidden[b, j] W[i, j]  (Wh half of Wt)
                q_ps = setup_ps.tile([128, HC, BL], F32, tag="qps")
                for ic in range(HC):
                    for jc in range(HC):
                        nc.tensor.matmul(
                            q_ps[:, ic, :],
                            wt_sb[:, jc, bass.ts(ic, 128)],
                            ht_sb[:, jc, :],
                            start=(jc == 0),
                            stop=(jc == HC - 1),
                        )
                # qb = q + bias  (per-partition bias add on ACT)
                for ic in range(HC):
                    nc.scalar.activation(
                        qb_sb[:, ic, :],
                        q_ps[:, ic, :],
                        mybir.ActivationFunctionType.Identity,
                        bias=b_sb[:, ic : ic + 1],
                    )

            # ---- main pools ----
            with (
                tc.tile_pool(name="xn", bufs=nb("xn", ns)) as xn_pool,
                tc.tile_pool(name="xt", bufs=nb("xt", 3)) as xt_pool,
                tc.tile_pool(name="tanh", bufs=nb("tanh", 6)) as tanh_pool,
                tc.tile_pool(name="small", bufs=nb("small", 3)) as small_pool,
                tc.tile_pool(
                    name="eps", bufs=nb("eps", 2), space=bass.MemorySpace.PSUM
                ) as e_ps_pool,
                tc.tile_pool(
                    name="sps", bufs=nb("sps", 2), space=bass.MemorySpace.PSUM
                ) as s_ps_pool,
                tc.tile_pool(
                    name="cps", bufs=nb("cps", 2), space=bass.MemorySpace.PSUM
                ) as c_ps_pool,
            ):

                def load_slab(s):
                    """Issue the two DMA loads for slab s (1024 t rows,
                    both batches).  Returns (xn_s, xt_s) tiles."""
                    t0 = s * TB
                    # natural layout, gpsimd SWDGE ring (2MB) - only the
                    # ctx matmuls read this, at the very end
                    xn_s = xn_pool.tile(
                        [128, TB // 128, BL, H], BF16, tag="xn",
                        name=f"xn{s}",
                    )
                    if on("load"):
                        nc.gpsimd.dma_start(
                            xn_s[:],
                            enc_d[t0 : t0 + TB, :, :].rearrange(
                                "(n p) b h -> p n b h", p=128
                            ),
                        )
                    # transposed layout, sync HWDGE ring (1MB per batch;
                    # split per batch to keep the DMA APs at 3 dims) -
                    # feeds the energy matmuls (PE critical path)
                    xt_s = xt_pool.tile(
                        [128, HC, BL, TB], BF16, tag="xt", name=f"xt{s}"
                    )
                    if on("load"):
                        enct_r = enct_d.rearrange(
                            "(jc p) b t -> p jc b t", p=128
                        )
                        for bi in range(BL):
                            nc.sync.dma_start(
                                xt_s[:, :, bi, :],
                                enct_r[:, :, bi, t0 : t0 + TB],
                            )
                    return xn_s, xt_s

                def score_block(xt_s, srow, sums, s, bi):
                    """Energy + tanh + v-dot + exp for one (slab, batch):
                    1024 t positions."""
                    if not on("energy"):
                        return
                    ths = []
                    for ic in range(HC):
                        e_ps = e_ps_pool.tile([128, TB], F32, tag="eps")
                        for jc in range(HC):
                            for half in range(2):
                                sl = bass.ts(half, 512)
                                nc.tensor.matmul(
                                    e_ps[:, sl],
                                    wet_bf[:, jc, bass.ts(ic, 128)],
                                    xt_s[:, jc, bi, sl],
                                    start=(jc == 0),
                                    stop=(jc == HC - 1),
                                )
                        if not on("tanh"):
                            continue
                        th = tanh_pool.tile(
                            [128, TB], BF16, tag="th", name=f"th{ic}"
                        )
                        nc.scalar.activation(
                            th[:],
                            e_ps[:],
                            mybir.ActivationFunctionType.Tanh,
                            bias=qb_sb[:, ic, bi : bi + 1],
                        )
                        ths.append(th)
                    if not on("vdot"):
                        return
                    s_ps = [
                        s_ps_pool.tile([1, 512], F32, tag="sps", name=f"sps{h}")
                        for h in range(2)
                    ]
                    for ic in range(HC):
                        for half in range(2):
                            sl = bass.ts(half, 512)
                            nc.tensor.matmul(
                                s_ps[half][:],
                                v_bf[:, ic : ic + 1],
                                ths[ic][:, sl],
                                start=(ic == 0),
                                stop=(ic == HC - 1),
                            )
                    for half in range(2):
                        blk = s * 2 + half
                        nc.scalar.activation(
                            srow[0:1, bass.ts(blk, 512)],
                            s_ps[half][:],
                            mybir.ActivationFunctionType.Exp,
                            accum_out=sums[0:1, blk : blk + 1],
                        )

                def finish_softmax(srow, sums):
                    attn_t = small_pool.tile([128, nblk * 4, 16], BF16,
                                             tag="attn")
                    nc.sync.dma_start(attn_t[:], srow[:], transpose=True)
                    s_tot = small_pool.tile([1, 1], F32, tag="stot")
                    nc.vector.reduce_sum(
                        s_tot[:], sums[:], axis=mybir.AxisListType.X
                    )
                    r_s = small_pool.tile([1, 1], F32, tag="rs")
                    nc.vector.reciprocal(r_s[:], s_tot[:])
                    return attn_t, r_s

                def ctx_phase(xn_tiles, attn_t, r_s, bi):
                    c_ps = c_ps_pool.tile([1, H], F32, tag="cps")
                    nchunks = ns * (TB // 128)
                    for c in range(nchunks):
                        nc.tensor.matmul(
                            c_ps[:],
                            attn_t[:, c, 0:1],
                            xn_tiles[c // 8][:, c % 8, bi, :],
                            start=(c == 0),
                            stop=(c == nchunks - 1),
                        )
                    ctx_sb = small_pool.tile([1, H], F32, tag="ctx")
                    nc.vector.tensor_scalar_mul(ctx_sb[:], c_ps[:], r_s[:])
                    nc.sync.dma_start(out_d[:, bi, :], ctx_sb[:])

                def main_body():
                    for _ in range(n_iters):
                        sums_l = [
                            small_pool.tile(
                                [1, nblk], F32, tag="sums", name=f"sums{i}"
                            )
                            for i in range(BL)
                        ]
                        xn_tiles = []
                        for s in range(ns):
                            xn_s, xt_s = load_slab(s)
                            xn_tiles.append(xn_s)
                            for bi in range(BL):
                                score_block(
                                    xt_s, srow_p[bi], sums_l[bi], s, bi
                                )
                        if not on("finish"):
                            continue
                        # batch-0 softmax finish + ctx interleave with
                        # nothing (tail); attn transposes ride the now
                        # idle sync ring
                        at0, rs0 = finish_softmax(srow_p[0], sums_l[0])
                        ctx_phase(xn_tiles, at0, rs0, 0)
                        at1, rs1 = finish_softmax(srow_p[1], sums_l[1])
                        ctx_phase(xn_tiles, at1, rs1, 1)

                if loop_iters > 0:
                    with tc.For_i(0, loop_iters, 1):
                        main_body()
                else:
                    main_body()

    nc.compile()
    return nc


# ---------------------------------------------------------------------------
# Host side: shard, run on 8 cores via PJRT (axon), gather.
# ---------------------------------------------------------------------------


class _SpmdRunner:
    """Build the sharded jit once; reuse across calls."""

    def __init__(self, nc, n_cores):
        import jax
        from jax.sharding import Mesh, PartitionSpec
        from jax.experimental.shard_map import shard_map
        from concourse.bass2jax import (
            _bass_exec_p,
            install_neuronx_cc_hook,
            partition_id_tensor,
        )

        install_neuronx_cc_hook()
        self.jax = jax
        self.n_cores = n_cores
        partition_name = (
            nc.partition_id_tensor.name if nc.partition_id_tensor else None
        )
        in_names, out_names, out_avals, zero_outs = [], [], [], []
        for alloc in nc.m.functions[0].allocations:
            if not isinstance(alloc, mybir.MemoryLocationSet):
                continue
            name = alloc.memorylocations[0].name
            if alloc.kind == "ExternalInput":
                if name != partition_name:
                    in_names.append(name)
            elif alloc.kind == "ExternalOutput":
                shape = tuple(alloc.tensor_shape)
                dtype = mybir.dt.np(alloc.dtype)
                out_names.append(name)
                out_avals.append(jax.core.ShapedArray(shape, dtype))
                zero_outs.append(np.zeros(shape, dtype))
        self.in_names = list(in_names)
        self.out_names = out_names
        self.out_avals = out_avals
        n_params = len(in_names)
        n_outs = len(out_avals)
        all_in_names = in_names + out_names
        if partition_name is not None:
            all_in_names.append(partition_name)

        def _body(*args):
            operands = list(args)
            if partition_name is not None:
                operands.append(partition_id_tensor())
            outs = _bass_exec_p.bind(
                *operands,
                out_avals=tuple(out_avals),
                in_names=tuple(all_in_names),
                out_names=tuple(out_names),
                lowering_input_output_aliases=(),
                sim_require_finite=True,
                sim_require_nnan=True,
                nc=nc,
            )
            return tuple(outs)

        devices = jax.devices()[:n_cores]
        assert len(devices) == n_cores, (
            f"need {n_cores} neuron cores, found {len(jax.devices())}"
        )
        self.mesh = Mesh(np.asarray(devices), ("core",))
        in_specs = (PartitionSpec("core"),) * (n_params + n_outs)
        out_specs = (PartitionSpec("core"),) * n_outs
        self.fn = jax.jit(
            shard_map(
                _body,
                mesh=self.mesh,
                in_specs=in_specs,
                out_specs=out_specs,
                check_rep=False,
            ),
            keep_unused=True,
        )
        self._sharding = jax.sharding.NamedSharding(
            self.mesh, PartitionSpec("core")
        )
        self._dev_zeros = [
            jax.device_put(
                np.zeros((n_cores * z.shape[0], *z.shape[1:]), z.dtype),
                self._sharding,
            )
            for z in zero_outs
        ]

    def put_inputs(self, in_maps):
        n = self.n_cores
        concat_in = [
            np.concatenate(
                [np.asarray(in_maps[c][name]) for c in range(n)], axis=0
            )
            for name in self.in_names
        ]
        return [self.jax.device_put(x, self._sharding) for x in concat_in]

    def run_device(self, dev_in):
        outs = self.fn(*dev_in, *self._dev_zeros)
        self.jax.block_until_ready(outs)
        return outs

    def run(self, in_maps):
        outs = self.run_device(self.put_inputs(in_maps))
        n = self.n_cores
        return [
            {
                name: np.asarray(outs[i]).reshape(
                    n, *self.out_avals[i].shape
                )[c]
                for i, name in enumerate(self.out_names)
            }
            for c in range(n)
        ]


def shard_inputs(hidden, encoder_outputs, W, b, v):
    """Per-core input dicts. The big tensor is sliced, cast to bf16 and
    staged twice (natural [T,BL,H] + transposed [H,BL,T]); small tensors
    are layout-packed for 128-partition SBUF residency."""
    hidden = np.asarray(hidden, dtype=np.float32)
    enc = np.asarray(encoder_outputs, dtype=np.float32)
    W = np.asarray(W, dtype=np.float32)
    b = np.asarray(b, dtype=np.float32)
    v = np.asarray(v, dtype=np.float32)

    wt = np.ascontiguousarray(W.T)  # [2H, H]
    b_packed = np.ascontiguousarray(b.reshape(HC, 128).T)  # [128, HC]
    v_packed = np.ascontiguousarray(v.reshape(HC, 128).T)  # [128, HC]
    enc_bf = enc.astype(ml_dtypes.bfloat16)  # [T, B, H]
    in_maps = []
    for c in range(N_CORES):
        bsl = slice(c * BL, (c + 1) * BL)
        h_l = hidden[0, bsl, :]  # [BL, H]
        # ht_packed[p, jc, b] = hidden[b, jc*128 + p]
        ht_packed = np.ascontiguousarray(
            h_l.T.reshape(HC, 128, BL).transpose(1, 0, 2)
        )
        enc_sl = enc_bf[:, bsl, :]  # [T, BL, H]
        in_maps.append(
            {
                "enc": np.ascontiguousarray(enc_sl),
                "enct": np.ascontiguousarray(enc_sl.transpose(2, 1, 0)),
                "wt": wt,
                "ht": ht_packed,
                "bias": b_packed,
                "v": v_packed,
            }
        )
    return in_maps


_RUNNER = None


def _get_runner():
    global _RUNNER
    if _RUNNER is None:
        _RUNNER = _SpmdRunner(build_nc(), N_CORES)
    return _RUNNER


def kernel(hidden, encoder_outputs, W, b, v):
    runner = _get_runner()
    in_maps = shard_inputs(hidden, encoder_outputs, W, b, v)
    res = runner.run(in_maps)
    out = np.concatenate([r["out"] for r in res], axis=1)  # [1, 16, 512]
    return out.astype(np.float32)


# revision 15
# speedup vs baseline: 1.9081x; 1.0015x over previous
"""Bahdanau (additive) attention kernel for 8 Trainium2 NeuronCores.

Problem:  hidden [1,16,512], encoder_outputs [4096,16,512], W [512,1024],
          b [512], v [512]  ->  ctx [1,16,512]
  energy = tanh(hidden @ Wh.T + enc @ We.T + b); scores = energy @ v
  attn = softmax_T(scores); ctx = sum_t attn * enc
  rel-err budget 2e-2; all-bf16 pipeline measures 2.3e-3.

Sharding: data-parallel over batch B=16 -> 2 batches per core, no comm.

Key layout decisions (v3):
  - enc is cast to bf16 ON HOST and staged in DRAM TWICE: natural
    [T, BL, H] (feeds the ctx matmul, which contracts over t) and
    pre-transposed [H, BL, T] (feeds the energy matmul, which contracts
    over h).  This removes the on-device xbar transpose, which measured
    68us/iter and serialized against the HBM loads.  Total HBM traffic
    16.8MB/core (~47us at 358GB/s) stays under the bf16 PE floor (82us),
    so the kernel is PE-bound as it should be.
  - transposed loads go on the sync HWDGE ring (PE-critical path);
    natural loads trickle on the gpsimd SWDGE ring (only needed by the
    ctx matmuls at the very end).
  - energyT psum tiles [128 h_out, 1024 t] = WeT.T @ XT  (PE, bf16,
    1024-col moving operand = max for bf16)
  - tanh on ACT with per-partition bias = (hidden @ Wh.T + b)  (folded)
  - scores = v.T @ tanhE on PE -> psum [1, 512] blocks
  - exp on ACT (unnormalized softmax; fp32 range is safe since
    |scores| <= ||v||_1 ~ 18) with fused accum_out giving block sums
  - attn row [1,T] -> [128 t, 32] layout via padded xbar transpose
  - ctx_raw = sum_t attn_t * X_t on PE (natural layout), then scale by
    1/sum(exp) on DVE, DMA out.
"""

import sys

if "/opt/trn_rl_repo" not in sys.path:
    sys.path.insert(0, "/opt/trn_rl_repo")

import ml_dtypes
import numpy as np

import concourse.bass as bass
import concourse.tile as tile
from concourse import bacc, mybir

F32 = mybir.dt.float32
BF16 = mybir.dt.bfloat16

N_CORES = 8
T = 4096
B = 16
H = 512
BL = B // N_CORES  # batches per core = 2
HC = H // 128  # h chunks = 4
TB = 1024  # t macro-block (slab)
NC32 = T // 128  # 32 t-chunks of 128 per batch


def build_nc(
    n_iters: int = 1,
    debug: bool = False,
    t_dim: int = T,
    loop_iters: int = 0,
    bufs: dict | None = None,
    # bisection knob: subset of {"load", "energy", "tanh", "vdot",
    # "finish"} - later stages require earlier ones. None = all.
    stages: frozenset | None = None,
    **_unused,
):
    """Build the per-core Bass program. n_iters>1 unrolls the whole main
    body; loop_iters>0 additionally wraps the body in a hardware For_i
    loop (for wall-clock benchmarking -- the axon transport adds tens of
    ms of jitter, so device time must dominate)."""
    bufs = dict(bufs or {})

    def nb(name, default):
        return bufs.get(name, default)

    t = t_dim
    ns = t // TB  # number of slabs
    nblk = t // 512  # exp blocks per batch

    nc = bacc.Bacc(
        "TRN2", target_bir_lowering=False, debug=debug, num_devices=N_CORES
    )
    enc_d = nc.declare_dram_parameter("enc", [t, BL, H], BF16, isOutput=False)
    enct_d = nc.declare_dram_parameter(
        "enct", [BL, t // TB, H, TB], BF16, isOutput=False
    )
    wt_d = nc.declare_dram_parameter("wt", [2 * H, H], F32, isOutput=False)
    ht_d = nc.declare_dram_parameter("ht", [128, HC, BL], F32, isOutput=False)
    b_d = nc.declare_dram_parameter("bias", [128, HC], F32, isOutput=False)
    v_d = nc.declare_dram_parameter("v", [128, HC], F32, isOutput=False)
    out_d = nc.declare_dram_parameter("out", [1, BL, H], F32, isOutput=True)

    ST = stages

    def on(s):
        return ST is None or s in ST

    with tile.TileContext(nc) as tc:
        with tc.tile_pool(name="persist", bufs=1) as persist:
            # ---- one-time setup: weights, query bias ----
            wet_bf = persist.tile([128, HC, H], BF16, tag="wet")
            qb_sb = persist.tile([128, HC, BL], F32, tag="qb")
            v_bf = persist.tile([128, HC], BF16, tag="vbf")
            srow_p = [
                persist.tile([16, t], BF16, tag=f"srow{i}", name=f"srow{i}")
                for i in range(BL)
            ]
            for i in range(BL):
                nc.gpsimd.memset(srow_p[i][:], 0.0)
            with (
                tc.tile_pool(name="setup", bufs=1) as setup,
                tc.tile_pool(
                    name="setup_ps", bufs=1, space=bass.MemorySpace.PSUM
                ) as setup_ps,
            ):
                # setup loads go on the ACT HWDGE ring so the first enc
                # loads aren't queued behind them on the sync ring; Wh
                # half first so the q-matmul (first in the PE stream)
                # unblocks after 1MB instead of 2MB
                ht_sb = setup.tile([128, HC, BL], F32, tag="ht")
                nc.scalar.dma_start(ht_sb[:], ht_d[:])
                b_sb = setup.tile([128, HC], F32, tag="bias")
                nc.scalar.dma_start(b_sb[:], b_d[:])
                v_f32 = setup.tile([128, HC], F32, tag="vf")
                nc.scalar.dma_start(v_f32[:], v_d[:])
                wt_sb = setup.tile([128, 2 * HC, H], F32, tag="wt")
                wt_r = wt_d.rearrange("(c p) i -> p c i", p=128)
                nc.scalar.dma_start(wt_sb[:, :HC, :], wt_r[:, :HC, :])
                nc.scalar.dma_start(wt_sb[:, HC:, :], wt_r[:, HC:, :])
                nc.vector.tensor_copy(v_bf[:], v_f32[:])
                for jc in range(HC):
                    nc.vector.tensor_copy(
                        wet_bf[:, jc, :], wt_sb[:, HC + jc, :]
                    )
                # q[i, b] = sum_j hidden[b, j] W[i, j]  (Wh half of Wt)
                q_ps = setup_ps.tile([128, HC, BL], F32, tag="qps")
                for ic in range(HC):
                    for jc in range(HC):
                        nc.tensor.matmul(
                            q_ps[:, ic, :],
                            wt_sb[:, jc, bass.ts(ic, 128)],
                            ht_sb[:, jc, :],
                            start=(jc == 0),
                            stop=(jc == HC - 1),
                        )
                # qb = q + bias  (per-partition bias add on ACT)
                for ic in range(HC):
                    nc.scalar.activation(
                        qb_sb[:, ic, :],
                        q_ps[:, ic, :],
                        mybir.ActivationFunctionType.Identity,
                        bias=b_sb[:, ic : ic + 1],
                    )

            # ---- main pools ----
            with (
                tc.tile_pool(name="xn", bufs=nb("xn", ns)) as xn_pool,
                tc.tile_pool(name="xt", bufs=nb("xt", 3)) as xt_pool,
                tc.tile_pool(name="tanh", bufs=nb("tanh", 6)) as tanh_pool,
                tc.tile_pool(name="small", bufs=nb("small", 3)) as small_pool,
                tc.tile_pool(
                    name="eps", bufs=nb("eps", 2), space=bass.MemorySpace.PSUM
                ) as e_ps_pool,
                tc.tile_pool(
                    name="sps", bufs=nb("sps", 2), space=bass.MemorySpace.PSUM
                ) as s_ps_pool,
                tc.tile_pool(
                    name="cps", bufs=nb("cps", 2), space=bass.MemorySpace.PSUM
                ) as c_ps_pool,
            ):

                def load_slab(s):
                    """Issue the two DMA loads for slab s (1024 t rows,
                    both batches).  Returns (xn_s, xt_s) tiles."""
                    t0 = s * TB
                    # natural layout, gpsimd SWDGE ring (2MB) - only the
                    # ctx matmuls read this, at the very end
                    xn_s = xn_pool.tile(
                        [128, TB // 128, BL, H], BF16, tag="xn",
                        name=f"xn{s}",
                    )
                    if on("load"):
                        nc.gpsimd.dma_start(
                            xn_s[:],
                            enc_d[t0 : t0 + TB, :, :].rearrange(
                                "(n p) b h -> p n b h", p=128
                            ),
                        )
                    # transposed layout, sync HWDGE ring (one contiguous
                    # 1MB DMA per batch; slab-major host layout) - feeds
                    # the energy matmuls (PE critical path)
                    xt_s = xt_pool.tile(
                        [128, HC, BL, TB], BF16, tag="xt", name=f"xt{s}"
                    )
                    if on("load"):
                        for bi in range(BL):
                            nc.sync.dma_start(
                                xt_s[:, :, bi, :],
                                enct_d[bi, s].rearrange(
                                    "(jc p) t -> p jc t", p=128
                                ),
                            )
                    return xn_s, xt_s

                def score_block(xt_s, srow, sums, s, bi):
                    """Energy + tanh + v-dot + exp for one (slab, batch):
                    1024 t positions."""
                    if not on("energy"):
                        return
                    ths = []
                    for ic in range(HC):
                        e_ps = e_ps_pool.tile([128, TB], F32, tag="eps")
                        for jc in range(HC):
                            for half in range(2):
                                sl = bass.ts(half, 512)
                                nc.tensor.matmul(
                                    e_ps[:, sl],
                                    wet_bf[:, jc, bass.ts(ic, 128)],
                                    xt_s[:, jc, bi, sl],
                                    start=(jc == 0),
                                    stop=(jc == HC - 1),
                                )
                        if not on("tanh"):
                            continue
                        th = tanh_pool.tile(
                            [128, TB], BF16, tag="th", name=f"th{ic}"
                        )
                        nc.scalar.activation(
                            th[:],
                            e_ps[:],
                            mybir.ActivationFunctionType.Tanh,
                            bias=qb_sb[:, ic, bi : bi + 1],
                        )
                        ths.append(th)
                    if not on("vdot"):
                        return
                    s_ps = [
                        s_ps_pool.tile([1, 512], F32, tag="sps", name=f"sps{h}")
                        for h in range(2)
                    ]
                    for ic in range(HC):
                        for half in range(2):
                            sl = bass.ts(half, 512)
                            nc.tensor.matmul(
                                s_ps[half][:],
                                v_bf[:, ic : ic + 1],
                                ths[ic][:, sl],
                                start=(ic == 0),
                                stop=(ic == HC - 1),
                            )
                    for half in range(2):
                        blk = s * 2 + half
                        nc.scalar.activation(
                            srow[0:1, bass.ts(blk, 512)],
                            s_ps[half][:],
                            mybir.ActivationFunctionType.Exp,
                            accum_out=sums[0:1, blk : blk + 1],
                        )

                def finish_softmax(srow, sums):
                    # scalar HWDGE ring: keeps the sync ring free so the
                    # next iteration's transposed loads prefetch during
                    # the ctx tail
                    attn_t = small_pool.tile([128, nblk * 4, 16], BF16,
                                             tag="attn")
                    nc.scalar.dma_start(attn_t[:], srow[:], transpose=True)
                    s_tot = small_pool.tile([1, 1], F32, tag="stot")
                    nc.vector.reduce_sum(
                        s_tot[:], sums[:], axis=mybir.AxisListType.X
                    )
                    r_s = small_pool.tile([1, 1], F32, tag="rs")
                    nc.vector.reciprocal(r_s[:], s_tot[:])
                    return attn_t, r_s

                def ctx_phase(xn_tiles, attn_t, r_s, bi):
                    c_ps = c_ps_pool.tile([1, H], F32, tag="cps")
                    nchunks = ns * (TB // 128)
                    for c in range(nchunks):
                        nc.tensor.matmul(
                            c_ps[:],
                            attn_t[:, c, 0:1],
                            xn_tiles[c // 8][:, c % 8, bi, :],
                            start=(c == 0),
                            stop=(c == nchunks - 1),
                        )
                    ctx_sb = small_pool.tile([1, H], F32, tag="ctx")
                    nc.vector.tensor_scalar_mul(ctx_sb[:], c_ps[:], r_s[:])
                    nc.scalar.dma_start(out_d[:, bi, :], ctx_sb[:])

                def main_body():
                    for _ in range(n_iters):
                        sums_l = [
                            small_pool.tile(
                                [1, nblk], F32, tag="sums", name=f"sums{i}"
                            )
                            for i in range(BL)
                        ]
                        xn_tiles = []
                        for s in range(ns):
                            xn_s, xt_s = load_slab(s)
                            xn_tiles.append(xn_s)
                            for bi in range(BL):
                                score_block(
                                    xt_s, srow_p[bi], sums_l[bi], s, bi
                                )
                        if not on("finish"):
                            continue
                        # both attn transposes first (they pipeline
                        # back-to-back on the scalar ring), then the two
                        # ctx matmul phases
                        at0, rs0 = finish_softmax(srow_p[0], sums_l[0])
                        at1, rs1 = finish_softmax(srow_p[1], sums_l[1])
                        ctx_phase(xn_tiles, at0, rs0, 0)
                        ctx_phase(xn_tiles, at1, rs1, 1)

                if loop_iters > 0:
                    with tc.For_i(0, loop_iters, 1):
                        main_body()
                else:
                    main_body()

    nc.compile()
    return nc


# ---------------------------------------------------------------------------
# Host side: shard, run on 8 cores via PJRT (axon), gather.
# ---------------------------------------------------------------------------


class _SpmdRunner:
    """Build the sharded jit once; reuse across calls."""

    def __init__(self, nc, n_cores):
        import jax
        from jax.sharding import Mesh, PartitionSpec
        from jax.experimental.shard_map import shard_map
        from concourse.bass2jax import (
            _bass_exec_p,
            install_neuronx_cc_hook,
            partition_id_tensor,
        )

        install_neuronx_cc_hook()
        self.jax = jax
        self.n_cores = n_cores
        partition_name = (
            nc.partition_id_tensor.name if nc.partition_id_tensor else None
        )
        in_names, out_names, out_avals, zero_outs = [], [], [], []
        for alloc in nc.m.functions[0].allocations:
            if not isinstance(alloc, mybir.MemoryLocationSet):
                continue
            name = alloc.memorylocations[0].name
            if alloc.kind == "ExternalInput":
                if name != partition_name:
                    in_names.append(name)
            elif alloc.kind == "ExternalOutput":
                shape = tuple(alloc.tensor_shape)
                dtype = mybir.dt.np(alloc.dtype)
                out_names.append(name)
                out_avals.append(jax.core.ShapedArray(shape, dtype))
                zero_outs.append(np.zeros(shape, dtype))
        self.in_names = list(in_names)
        self.out_names = out_names
        self.out_avals = out_avals
        n_params = len(in_names)
        n_outs = len(out_avals)
        all_in_names = in_names + out_names
        if partition_name is not None:
            all_in_names.append(partition_name)

        def _body(*args):
            operands = list(args)
            if partition_name is not None:
                operands.append(partition_id_tensor())
            outs = _bass_exec_p.bind(
                *operands,
                out_avals=tuple(out_avals),
                in_names=tuple(all_in_names),
                out_names=tuple(out_names),
                lowering_input_output_aliases=(),
                sim_require_finite=True,
                sim_require_nnan=True,
                nc=nc,
            )
            return tuple(outs)

        devices = jax.devices()[:n_cores]
        assert len(devices) == n_cores, (
            f"need {n_cores} neuron cores, found {len(jax.devices())}"
        )
        self.mesh = Mesh(np.asarray(devices), ("core",))
        in_specs = (PartitionSpec("core"),) * (n_params + n_outs)
        out_specs = (PartitionSpec("core"),) * n_outs
        self.fn = jax.jit(
            shard_map(
                _body,
                mesh=self.mesh,
                in_specs=in_specs,
                out_specs=out_specs,
                check_rep=False,
            ),
            keep_unused=True,
        )
        self._sharding = jax.sharding.NamedSharding(
            self.mesh, PartitionSpec("core")
        )
        self._dev_zeros = [
            jax.device_put(
                np.zeros((n_cores * z.shape[0], *z.shape[1:]), z.dtype),
                self._sharding,
            )
            for z in zero_outs
        ]

    def put_inputs(self, in_maps):
        n = self.n_cores
        concat_in = [
            np.concatenate(
                [np.asarray(in_maps[c][name]) for c in range(n)], axis=0
            )
            for name in self.in_names
        ]
        return [self.jax.device_put(x, self._sharding) for x in concat_in]

    def run_device(self, dev_in):
        outs = self.fn(*dev_in, *self._dev_zeros)
        self.jax.block_until_ready(outs)
        return outs

    def run(self, in_maps):
        outs = self.run_device(self.put_inputs(in_maps))
        n = self.n_cores
        return [
            {
                name: np.asarray(outs[i]).reshape(
                    n, *self.out_avals[i].shape
                )[c]
                for i, name in enumerate(self.out_names)
            }
            for c in range(n)
        ]


def shard_inputs(hidden, encoder_outputs, W, b, v):
    """Per-core input dicts. The big tensor is sliced, cast to bf16 and
    staged twice (natural [T,BL,H] + transposed [H,BL,T]); small tensors
    are layout-packed for 128-partition SBUF residency."""
    hidden = np.asarray(hidden, dtype=np.float32)
    enc = np.asarray(encoder_outputs, dtype=np.float32)
    W = np.asarray(W, dtype=np.float32)
    b = np.asarray(b, dtype=np.float32)
    v = np.asarray(v, dtype=np.float32)

    wt = np.ascontiguousarray(W.T)  # [2H, H]
    b_packed = np.ascontiguousarray(b.reshape(HC, 128).T)  # [128, HC]
    v_packed = np.ascontiguousarray(v.reshape(HC, 128).T)  # [128, HC]
    enc_bf = enc.astype(ml_dtypes.bfloat16)  # [T, B, H]
    in_maps = []
    for c in range(N_CORES):
        bsl = slice(c * BL, (c + 1) * BL)
        h_l = hidden[0, bsl, :]  # [BL, H]
        # ht_packed[p, jc, b] = hidden[b, jc*128 + p]
        ht_packed = np.ascontiguousarray(
            h_l.T.reshape(HC, 128, BL).transpose(1, 0, 2)
        )
        enc_sl = enc_bf[:, bsl, :]  # [T, BL, H]
        # slab-major transposed layout [BL, ns, H, TB]
        enct = np.ascontiguousarray(
            enc_sl.reshape(T // TB, TB, BL, H).transpose(2, 0, 3, 1)
        )
        in_maps.append(
            {
                "enc": np.ascontiguousarray(enc_sl),
                "enct": enct,
                "wt": wt,
                "ht": ht_packed,
                "bias": b_packed,
                "v": v_packed,
            }
        )
    return in_maps


_RUNNER = None


def _get_runner():
    global _RUNNER
    if _RUNNER is None:
        _RUNNER = _SpmdRunner(build_nc(), N_CORES)
    return _RUNNER


def kernel(hidden, encoder_outputs, W, b, v):
    runner = _get_runner()
    in_maps = shard_inputs(hidden, encoder_outputs, W, b, v)
    res = runner.run(in_maps)
    out = np.concatenate([r["out"] for r in res], axis=1)  # [1, 16, 512]
    return out.astype(np.float32)
